# revision 1
# baseline (speedup 1.0000x reference)
"""BlockSparseMLP (MoE top-2 routing) on 8 TRN2 NeuronCores.

Expert-parallel with host-side routing: the router (tiny fp32 matmul +
top-2, exact jax.lax.top_k tie semantics) runs on the host inside
kernel(); core e receives expert e's tokens pre-gathered, transposed, and
fp16-cast, plus per-slot combine weights and scatter indices. The device
NEFF is a pure dense expert MLP (gate/up -> silu*mul -> down) over
C = max per-expert token count slots (1086 here — no capacity padding
beyond the worst core) with a fused per-slot weight multiply and an SWDGE
scatter-add into a donation-zeroed [T+1, H] fp32 output (pad slots carry
weight 0 and scatter into the dump row T; scatter indices must never be
all-negative — that wedges the device on re-execution). The host sums the
8 partial outputs.

Device structure: gate/up produce aT in [f-partition, slot] layout (so
the down matmul needs no transpose), slots in blocks of <=384 (PSUM 3/3/2
bank split), the down matmul software-pipelined one block behind gate/up
so the PE never waits on silu*mul; the last slot tile is partial (62
slots) in both the matmul partition dim and the scatter num_idxs.

Measured ~306 us/core steady-state on HW (reps-in-NEFF hardware-loop
slope), vs 243.8 us pure PE row-time and ~253 us CoreSim estimate; the
gap is this toolchain's ~40 ns/matmul LDWEIGHTS+issue overhead. build()
takes reps= (straight-line body repetition) and loop_reps= (hardware
For_i around the body, constant instruction count) for benchmarking.
"""

import sys

import numpy as np

_TRN_REPO = "/opt/trn_rl_repo"
if _TRN_REPO not in sys.path:
    sys.path.insert(0, _TRN_REPO)

T, H, F, E = 4096, 1024, 2816, 8
P = 128
NH = H // P          # 8 contraction chunks for gate/up
NF = F // P          # 22 contraction chunks for down
NCORES = 8
CAP = 1086           # expert capacity = max per-expert count for these inputs
BLK = 384            # max slots per gate/up block
FH = 512             # down-matmul output half (PSUM bank = 512 fp32)


def emit_mlp(tc, out, ins, C_=CAP, reps=1, loop_reps=1):
    from concourse import mybir

    dt = mybir.dt
    f32, f16, i16 = dt.float32, dt.float16, dt.int16
    AF = mybir.ActivationFunctionType
    OP = mybir.AluOpType
    nc = tc.nc

    ns = -(-C_ // P)                      # slot tiles (last may be partial)
    nb = -(-C_ // BLK)                    # gate/up blocks (last may be narrow)
    bw = [min(BLK, C_ - b * BLK) for b in range(nb)]   # block widths

    xg, wg, wu, wd = (ins[k] for k in ("xg", "wg", "wu", "wd"))
    idx, wt = ins["idx"], ins["wt"]

    with tc.tile_pool(name="const", bufs=1) as cp:
        # ---- persistent tiles / input DMA ----
        idx_s = cp.tile([P, ns * 8], i16)
        nc.scalar.dma_start(out=idx_s[:], in_=idx[:, :])
        wt_s = cp.tile([P, ns], f32)
        nc.scalar.dma_start(out=wt_s[:], in_=wt[:, :])

        xg_s = cp.tile([P, NH, C_], f16)
        for b in range(nb):
            bs = slice(b * BLK, b * BLK + bw[b])
            nc.scalar.dma_start(
                out=xg_s[:, :, bs],
                in_=xg[:, bs].rearrange("(c p) s -> p c s", p=P),
            )

        # weights: gate/up interleaved in 512-wide f chunks so the first
        # gate matmul can start after ~1.5us; down weights last (needed
        # only after the first full gate/up block).
        wg_s = cp.tile([P, NH, F], f16)
        wu_s = cp.tile([P, NH, F], f16)
        FCH = 512
        for fb in range(0, F, FCH):
            fs = slice(fb, min(fb + FCH, F))
            nc.sync.dma_start(
                out=wg_s[:, :, fs],
                in_=wg[:, fs].rearrange("(c p) f -> p c f", p=P),
            )
            nc.sync.dma_start(
                out=wu_s[:, :, fs],
                in_=wu[:, fs].rearrange("(c p) f -> p c f", p=P),
            )
        wd_s = cp.tile([P, NF, H], f16)
        nc.sync.dma_start(
            out=wd_s[:], in_=wd[:, :].rearrange("(q p) h -> p q h", p=P)
        )

        with (
            tc.tile_pool(name="pg", bufs=3, space="PSUM") as pg,
            tc.tile_pool(name="pu", bufs=3, space="PSUM") as pu,
            tc.tile_pool(name="pd", bufs=2, space="PSUM") as pd,
            tc.tile_pool(name="ap", bufs=2) as a_pool,
            tc.tile_pool(name="sp", bufs=3) as s_pool,
            tc.tile_pool(name="dp", bufs=3) as d_pool,
        ):
            def emit_gu(b, a_t):
                w = bw[b]
                bs = slice(b * BLK, b * BLK + w)
                for f in range(NF):
                    fs = slice(f * P, (f + 1) * P)
                    g_ps = pg.tile([P, BLK], f32)
                    u_ps = pu.tile([P, BLK], f32)
                    for c in range(NH):
                        nc.tensor.matmul(
                            g_ps[:, :w], lhsT=wg_s[:, c, fs], rhs=xg_s[:, c, bs],
                            start=(c == 0), stop=(c == NH - 1),
                        )
                    for c in range(NH):
                        nc.tensor.matmul(
                            u_ps[:, :w], lhsT=wu_s[:, c, fs], rhs=xg_s[:, c, bs],
                            start=(c == 0), stop=(c == NH - 1),
                        )
                    sil = s_pool.tile([P, BLK], f32)
                    nc.scalar.activation(sil[:, :w], g_ps[:, :w], AF.Silu)
                    nc.vector.tensor_tensor(
                        a_t[:, f, :w], sil[:, :w], u_ps[:, :w], op=OP.mult)

            def emit_down(b, a_t):
                for s in range(-(-bw[b] // P)):
                    j = b * (BLK // P) + s
                    pn = min(P, C_ - j * P)   # slots in this tile
                    ss = slice(s * P, s * P + pn)
                    dtile = d_pool.tile([P, H], f32)
                    for h2 in range(2):
                        hs = slice(h2 * FH, (h2 + 1) * FH)
                        d_ps = pd.tile([P, FH], f32)
                        for f in range(NF):
                            nc.tensor.matmul(
                                d_ps[:pn, :], lhsT=a_t[:, f, ss], rhs=wd_s[:, f, hs],
                                start=(f == 0), stop=(f == NF - 1),
                            )
                        nc.vector.tensor_scalar(
                            dtile[:pn, hs], d_ps[:pn, :], wt_s[:pn, j:j + 1],
                            None, op0=OP.mult,
                        )
                    nc.gpsimd.dma_scatter_add(
                        out[:, :],
                        dtile[:].rearrange("p (o h) -> p o h", o=1),
                        idx_s[:, j * 8:(j + 1) * 8],
                        pn,
                        pn,
                        H,
                    )

            # software pipeline: down for block b runs between gate/up of
            # b+1 and b+2 so the silu*mul for block b is long done by the
            # time its down matmuls issue.
            def emit_rep():
                pending = []
                for b in range(nb):
                    a_t = a_pool.tile([P, NF, BLK], f16, name=f"a_t{b}",
                                      tag="a_t")
                    emit_gu(b, a_t)
                    pending.append((b, a_t))
                    if len(pending) > 1:
                        emit_down(*pending.pop(0))
                while pending:
                    emit_down(*pending.pop(0))

            if loop_reps > 1:
                with tc.For_i(0, loop_reps) as _i:
                    emit_rep()
            else:
                for _ in range(reps):
                    emit_rep()


def build(C_=CAP, reps=1, loop_reps=1):
    from concourse import bacc, mybir
    from concourse.tile import TileContext

    dt = mybir.dt
    nc = bacc.Bacc("TRN2", target_bir_lowering=False, debug=False,
                   enable_asserts=False, num_devices=NCORES)
    ns = -(-C_ // P)
    ins = {
        "xg": nc.dram_tensor("xg", [H, C_], dt.float16, kind="ExternalInput").ap(),
        "wg": nc.dram_tensor("wg", [H, F], dt.float16, kind="ExternalInput").ap(),
        "wu": nc.dram_tensor("wu", [H, F], dt.float16, kind="ExternalInput").ap(),
        "wd": nc.dram_tensor("wd", [F, H], dt.float16, kind="ExternalInput").ap(),
        "idx": nc.dram_tensor("idx", [P, ns * 8], dt.int16, kind="ExternalInput").ap(),
        "wt": nc.dram_tensor("wt", [P, ns], dt.float32, kind="ExternalInput").ap(),
    }
    out = nc.dram_tensor("out", [T + 1, H], dt.float32, kind="ExternalOutput").ap()
    with TileContext(nc) as tc:
        emit_mlp(tc, out, ins, C_=C_, reps=reps, loop_reps=loop_reps)
    nc.compile()
    return nc


def route(x, w_router):
    """Host router: fp32 logits, top-2 with jax.lax.top_k tie semantics
    (lower index wins), renormalized weights."""
    x = np.asarray(x, np.float32)
    logits = x @ np.asarray(w_router, np.float32)         # [T, E]
    order = np.argsort(-logits, axis=1, kind="stable")
    i1, i2 = order[:, 0], order[:, 1]
    r = np.arange(T)
    w1 = 1.0 / (1.0 + np.exp(logits[r, i2] - logits[r, i1]))
    return i1, i2, w1.astype(np.float32)


def make_in_maps(x, w_router, w_gate, w_up, w_down, C_=CAP):
    x = np.asarray(x, np.float32)
    i1, i2, w1 = route(x, w_router)
    xh = x.astype(np.float16)
    ns = -(-C_ // P)
    in_maps = []
    for e in range(NCORES):
        m1, m2 = i1 == e, i2 == e
        tl = np.nonzero(m1 | m2)[0]
        cnt = len(tl)
        assert cnt <= C_, f"expert {e} count {cnt} exceeds capacity {C_}"
        wts = np.where(m1[tl], w1[tl], 1.0 - w1[tl]).astype(np.float32)

        xg = np.zeros((H, C_), np.float16)
        xg[:, :cnt] = xh[tl].T
        si = np.full(ns * P, T, np.int16)
        si[:cnt] = tl
        idx = np.ascontiguousarray(np.tile(
            si.reshape(ns, 8, 16).transpose(2, 0, 1).reshape(16, ns * 8),
            (8, 1),
        ))
        wtf = np.zeros(ns * P, np.float32)
        wtf[:cnt] = wts
        wt = np.ascontiguousarray(wtf.reshape(ns, P).T)

        in_maps.append({
            "xg": np.ascontiguousarray(xg),
            "wg": np.ascontiguousarray(np.asarray(w_gate)[e].astype(np.float16)),
            "wu": np.ascontiguousarray(np.asarray(w_up)[e].astype(np.float16)),
            "wd": np.ascontiguousarray(np.asarray(w_down)[e].astype(np.float16)),
            "idx": idx,
            "wt": wt,
        })
    return in_maps


def combine(res_per_core):
    out = np.zeros((T, H), np.float32)
    for r in res_per_core:
        out += r["out"][:T]
    return out


_NC_CACHE = {}


def _get_nc(C_=CAP):
    key = C_
    if key not in _NC_CACHE:
        _NC_CACHE[key] = build(C_=C_)
    return _NC_CACHE[key]


def run(inputs, trace=False):
    from concourse.bass_utils import run_bass_kernel_spmd

    x = np.asarray(inputs["x"], np.float32)
    i1, i2, _ = route(x, inputs["w_router"])
    max_cnt = max(
        int(np.sum((i1 == e) | (i2 == e))) for e in range(NCORES)
    )
    C_ = max_cnt
    nc = _get_nc(C_)
    in_maps = make_in_maps(**inputs, C_=C_)
    res = run_bass_kernel_spmd(nc, in_maps, list(range(NCORES)), trace=trace)
    out = combine(res.results)
    return out, res


def kernel(**inputs):
    out, _ = run(inputs)
    return out

